# revision 4
# baseline (speedup 1.0000x reference)
"""Trainium2 Bass kernel for nn_EnvGenerator (K=8 parallel 2-layer edge MLPs
with Gumbel-sigmoid gating).

Strategy: data-parallel over edges across 8 NeuronCores; the small K MLP
weights are replicated on every core.

Per core (E_CORE = 25088 padded edges):
  - layer 1 on the PE array in native fp32 (4 cyc/row): for each 128-edge
    subtile, psum[128e, 512] = xT_chunk.T @ [W1_k | W1_k+1] accumulated over
    3 contraction chunks of 128 features; k-nets processed in pairs so each
    matmul streams the max fp32 moving size (N=512).
  - relu + *W2 + row-reduce fused into ONE vector-engine op
    (scalar_tensor_tensor: out=(psum max 0) * w2_rep, accum_out=row sum)
    accumulating logits columns per k.
  - gumbel-sigmoid gate: gate-noise (log terms) is precomputed on host in
    fp32; device adds logits, applies ACT sigmoid for the soft weight and a
    DVE is_ge for the hard threshold (sign of gate == sigmoid >= 0.5).

Inputs are laid out on the host so every DMA is partition-natural.
"""

import numpy as np

import concourse.bass as bass
import concourse.tile as tile
from concourse import bacc, mybir
from concourse.bass_utils import run_bass_kernel_spmd

F32 = mybir.dt.float32

N_CORES = 8
E = 200000
K = 8
F = 384  # in features  (3 chunks of 128)
M = 256  # mlp hidden
BIAS = 1e-4

BLK = 512                      # edges per DMA block
E_CORE = 25088                 # 49 * 512
E_PAD = E_CORE * N_CORES       # 200704
N_BLK = E_CORE // BLK          # 49
T_SUB = E_CORE // 128          # 196 subtiles of 128 edges per core

# set by test.py to capture a profiled run
TRACE = False
TRACE_DIR = None
LAST_RESULTS = None


def _build_program(b1_nonzero: bool):
    nc = bacc.Bacc(
        "TRN2", target_bir_lowering=False, debug=False, num_devices=N_CORES
    )

    xt_d = nc.dram_tensor("xt", [3, 128, E_CORE], F32, kind="ExternalInput").ap()
    w1_d = nc.dram_tensor("w1p", [3, 4, 128, 512], F32, kind="ExternalInput").ap()
    w2_d = nc.dram_tensor("w2r", [128, K * M], F32, kind="ExternalInput").ap()
    gn_d = nc.dram_tensor("gn", [K, 128, T_SUB], F32, kind="ExternalInput").ap()
    if b1_nonzero:
        b1_d = nc.dram_tensor("b1r", [128, K * M], F32, kind="ExternalInput").ap()
    ew_d = nc.dram_tensor("ew", [K, 128, T_SUB], F32, kind="ExternalOutput").ap()
    hd_d = nc.dram_tensor("hd", [K, 128, T_SUB], F32, kind="ExternalOutput").ap()

    with tile.TileContext(nc) as tc:
        with (
            tc.tile_pool(name="resident", bufs=1) as rpool,
            tc.tile_pool(name="x", bufs=3) as xpool,
            tc.tile_pool(name="scratch", bufs=2) as spool,
            tc.tile_pool(name="epi", bufs=2) as epool,
            tc.tile_pool(name="psum", bufs=8, space="PSUM") as pspool,
        ):
            # resident tensors
            w1_sb = rpool.tile([128, 3 * 4 * 512], F32)   # (c, pair) blocks of 512
            w2_sb = rpool.tile([128, K * M], F32)
            gn_sb = rpool.tile([128, K * T_SUB], F32)
            logits = rpool.tile([128, K * T_SUB], F32)
            for c in range(3):
                for pr in range(4):
                    nc.sync.dma_start(
                        w1_sb[:, (c * 4 + pr) * 512:(c * 4 + pr + 1) * 512],
                        w1_d[c, pr],
                    )
            nc.sync.dma_start(w2_sb[:], w2_d[:])
            for k in range(K):
                nc.sync.dma_start(
                    gn_sb[:, k * T_SUB:(k + 1) * T_SUB], gn_d[k]
                )
            if b1_nonzero:
                b1_sb = rpool.tile([128, K * M], F32)
                nc.sync.dma_start(b1_sb[:], b1_d[:])

            for blk in range(N_BLK):
                xt_t = xpool.tile([128, 3 * BLK], F32, tag="xt")
                for c in range(3):
                    nc.sync.dma_start(
                        xt_t[:, c * BLK:(c + 1) * BLK],
                        xt_d[c][:, blk * BLK:(blk + 1) * BLK],
                    )
                for j in range(BLK // 128):
                    ps = [
                        pspool.tile([128, 512], F32, tag="ps", name=f"ps{blk}_{j}_{pr}")
                        for pr in range(4)
                    ]
                    for c in range(3):
                        lhsT = xt_t[:, c * BLK + j * 128: c * BLK + (j + 1) * 128]
                        for pr in range(4):
                            nc.tensor.matmul(
                                ps[pr][:],
                                lhsT,
                                w1_sb[:, (c * 4 + pr) * 512:(c * 4 + pr + 1) * 512],
                                start=(c == 0),
                                stop=(c == 2),
                            )
                    t = blk * (BLK // 128) + j
                    for k in range(K):
                        pr, half = divmod(k, 2)
                        h_in = ps[pr][:, half * M:(half + 1) * M]
                        if b1_nonzero:
                            hb = spool.tile([128, M], F32, tag="hb")
                            nc.vector.tensor_tensor(
                                out=hb[:], in0=h_in,
                                in1=b1_sb[:, k * M:(k + 1) * M],
                                op=mybir.AluOpType.add,
                            )
                            h_in = hb[:]
                        sc = spool.tile([128, M], F32, tag="sc")
                        nc.vector.scalar_tensor_tensor(
                            out=sc[:], in0=h_in, scalar=0.0,
                            in1=w2_sb[:, k * M:(k + 1) * M],
                            op0=mybir.AluOpType.max,
                            op1=mybir.AluOpType.mult,
                            accum_out=logits[:, k * T_SUB + t: k * T_SUB + t + 1],
                        )

            # epilogue: gate -> sigmoid / hard threshold, per net
            for k in range(K):
                gate = epool.tile([128, T_SUB], F32, tag="gate")
                nc.vector.tensor_tensor(
                    out=gate[:],
                    in0=logits[:, k * T_SUB:(k + 1) * T_SUB],
                    in1=gn_sb[:, k * T_SUB:(k + 1) * T_SUB],
                    op=mybir.AluOpType.add,
                )
                ew_t = epool.tile([128, T_SUB], F32, tag="ew")
                nc.scalar.activation(
                    ew_t[:], gate[:], mybir.ActivationFunctionType.Sigmoid
                )
                hd_t = epool.tile([128, T_SUB], F32, tag="hd")
                nc.vector.tensor_scalar(
                    out=hd_t[:], in0=gate[:], scalar1=0.0, scalar2=None,
                    op0=mybir.AluOpType.is_ge,
                )
                nc.sync.dma_start(ew_d[k], ew_t[:])
                nc.sync.dma_start(hd_d[k], hd_t[:])

    nc.compile()
    return nc


def kernel(triplet_emb, noise, W1, b1, W2, b2, edge_index=None):
    global LAST_RESULTS
    triplet_emb = np.asarray(triplet_emb, dtype=np.float32)
    noise = np.asarray(noise, dtype=np.float32)
    W1 = np.asarray(W1, dtype=np.float32)
    b1 = np.asarray(b1, dtype=np.float32)
    W2 = np.asarray(W2, dtype=np.float32)
    b2 = np.asarray(b2, dtype=np.float32)

    # ---- host-side layout prep ----
    # gate noise (elementwise log terms, fp32 to match the reference) + b2
    eps = (1.0 - BIAS) - (1.0 - 2.0 * BIAS) * noise
    gn = (np.log(eps) - np.log1p(-eps)) + b2[:, None].astype(np.float32)
    gn = gn.astype(np.float32)
    gn_pad = np.zeros((K, E_PAD), np.float32)
    gn_pad[:, :E] = gn

    # W1 packed as [c, pair, 128f, 512] with nets 2pr|2pr+1 side by side
    w1p = np.ascontiguousarray(
        W1.reshape(4, 2, 3, 128, M).transpose(2, 0, 3, 1, 4).reshape(3, 4, 128, 512)
    )
    w2r = np.ascontiguousarray(np.broadcast_to(W2.reshape(1, K * M), (128, K * M)))
    b1_nonzero = bool(np.any(b1))
    b1r = (
        np.ascontiguousarray(np.broadcast_to(b1.reshape(1, K * M), (128, K * M)))
        if b1_nonzero
        else None
    )

    x_pad = np.zeros((E_PAD, F), np.float32)
    x_pad[:E] = triplet_emb

    in_maps = []
    for c in range(N_CORES):
        lo, hi = c * E_CORE, (c + 1) * E_CORE
        xt_c = np.ascontiguousarray(x_pad[lo:hi].T).reshape(3, 128, E_CORE)
        gn_c = np.ascontiguousarray(
            gn_pad[:, lo:hi].reshape(K, T_SUB, 128).transpose(0, 2, 1)
        )
        m = {"xt": xt_c, "w1p": w1p, "w2r": w2r, "gn": gn_c}
        if b1_nonzero:
            m["b1r"] = b1r
        in_maps.append(m)

    nc = _build_program(b1_nonzero)
    results = run_bass_kernel_spmd(
        nc, in_maps, list(range(N_CORES)), trace=TRACE, tmpdir=TRACE_DIR
    )
    LAST_RESULTS = results

    ew = np.empty((K, E_PAD), np.float32)
    hd = np.empty((K, E_PAD), np.float32)
    for c in range(N_CORES):
        lo, hi = c * E_CORE, (c + 1) * E_CORE
        ew[:, lo:hi] = results.results[c]["ew"].transpose(0, 2, 1).reshape(K, E_CORE)
        hd[:, lo:hi] = results.results[c]["hd"].transpose(0, 2, 1).reshape(K, E_CORE)

    edge_weight = np.ascontiguousarray(ew[:, :E])
    hard_edge_weight = np.ascontiguousarray(hd[:, :E])
    return edge_weight, hard_edge_weight


# revision 6
# speedup vs baseline: 3.4898x; 3.4898x over previous
"""Trainium2 Bass kernel for nn_EnvGenerator (K=8 parallel 2-layer edge MLPs
with Gumbel-sigmoid gating).

Strategy: data-parallel over edges across 8 NeuronCores; the small K MLP
weights are replicated on every core.

Per core (E_CORE = 25088 padded edges), fast path:
  - layer 1 on the PE array in float32r (1 cyc/row at N=512, measured
    233 ns/MM): for each 128-edge subtile, psum[128e, 512] =
    xT_chunk.T @ [W1_k | W1_k+1] accumulated over 3 chunks of 128 features;
    k-nets processed in pairs so each matmul streams N=512.
  - ACT relu PSUM->SBUF per psum tile, then one DVE tensor_tensor_reduce
    per net (mult by replicated W2, row-sum) accumulates logits columns.
  - gumbel-sigmoid gate: gate-noise (log terms) precomputed on host in
    fp32; device adds logits, ACT sigmoid for the soft weight, DVE is_ge
    for the hard threshold (sign of gate == sigmoid >= 0.5).
  - float32r is ~tf32 precision (logit err <= ~8e-4 abs), so the host
    recomputes the few edges with |edge_weight - 0.5| <= 1.5e-3 in exact
    fp32 (~1e-3 of all edges), making the hard outputs fp32-exact.

If the float32r run fails (or USE_F32R=False), falls back to a verified
native-fp32 program (4 cyc/row, fused DVE relu*W2+reduce off PSUM).

Inputs are laid out on the host so every DMA is partition-natural.
"""

import numpy as np

import concourse.bass as bass
import concourse.tile as tile
from concourse import bacc, mybir
from concourse.bass_utils import run_bass_kernel_spmd

F32 = mybir.dt.float32
F32R = mybir.dt.float32r

N_CORES = 8
E = 200000
K = 8
F = 384  # in features  (3 chunks of 128)
M = 256  # mlp hidden
BIAS = 1e-4

BLK = 512                      # edges per DMA block
E_CORE = 25088                 # 49 * 512
E_PAD = E_CORE * N_CORES       # 200704
N_BLK = E_CORE // BLK          # 49
T_SUB = E_CORE // 128          # 196 subtiles of 128 edges per core

USE_F32R = True
FIXUP_EW_DELTA = 1.5e-3        # |ew-0.5| below this -> exact fp32 recompute

# set by test.py to capture a profiled run
TRACE = False
TRACE_DIR = None
LAST_RESULTS = None


def _build_program(b1_nonzero: bool, use_f32r: bool):
    nc = bacc.Bacc(
        "TRN2", target_bir_lowering=False, debug=False, num_devices=N_CORES
    )
    DTM = F32R if use_f32r else F32

    xt_d = nc.dram_tensor("xt", [3, 128, E_CORE], DTM, kind="ExternalInput").ap()
    w1_d = nc.dram_tensor("w1p", [3, 4, 128, 512], DTM, kind="ExternalInput").ap()
    w2_d = nc.dram_tensor("w2r", [128, K * M], F32, kind="ExternalInput").ap()
    gn_d = nc.dram_tensor("gn", [K, 128, T_SUB], F32, kind="ExternalInput").ap()
    if b1_nonzero:
        b1_d = nc.dram_tensor("b1r", [128, K * M], F32, kind="ExternalInput").ap()
    ew_d = nc.dram_tensor("ew", [K, 128, T_SUB], F32, kind="ExternalOutput").ap()
    hd_d = nc.dram_tensor("hd", [K, 128, T_SUB], F32, kind="ExternalOutput").ap()

    with tile.TileContext(nc) as tc:
        with (
            tc.tile_pool(name="resident", bufs=1) as rpool,
            tc.tile_pool(name="x", bufs=3) as xpool,
            tc.tile_pool(name="relu", bufs=4) as relupool,
            tc.tile_pool(name="scratch", bufs=2) as spool,
            tc.tile_pool(name="epi", bufs=2) as epool,
            tc.tile_pool(name="psum", bufs=8, space="PSUM") as pspool,
        ):
            # resident tensors
            w1_sb = rpool.tile([128, 3 * 4 * 512], DTM)   # (c, pair) blocks of 512
            w2_sb = rpool.tile([128, K * M], F32)
            gn_sb = rpool.tile([128, K * T_SUB], F32)
            logits = rpool.tile([128, K * T_SUB], F32)
            for c in range(3):
                for pr in range(4):
                    nc.sync.dma_start(
                        w1_sb[:, (c * 4 + pr) * 512:(c * 4 + pr + 1) * 512],
                        w1_d[c, pr],
                    )
            nc.sync.dma_start(w2_sb[:], w2_d[:])
            for k in range(K):
                nc.sync.dma_start(
                    gn_sb[:, k * T_SUB:(k + 1) * T_SUB], gn_d[k]
                )
            if b1_nonzero:
                b1_sb = rpool.tile([128, K * M], F32)
                nc.sync.dma_start(b1_sb[:], b1_d[:])

            for blk in range(N_BLK):
                xt_t = xpool.tile([128, 3 * BLK], DTM, tag="xt")
                for c in range(3):
                    nc.sync.dma_start(
                        xt_t[:, c * BLK:(c + 1) * BLK],
                        xt_d[c][:, blk * BLK:(blk + 1) * BLK],
                    )
                for j in range(BLK // 128):
                    ps = [
                        pspool.tile([128, 512], F32, tag="ps", name=f"ps{blk}_{j}_{pr}")
                        for pr in range(4)
                    ]
                    for c in range(3):
                        lhsT = xt_t[:, c * BLK + j * 128: c * BLK + (j + 1) * 128]
                        for pr in range(4):
                            nc.tensor.matmul(
                                ps[pr][:],
                                lhsT,
                                w1_sb[:, (c * 4 + pr) * 512:(c * 4 + pr + 1) * 512],
                                start=(c == 0),
                                stop=(c == 2),
                            )
                    t = blk * (BLK // 128) + j

                    if use_f32r:
                        # ACT: relu PSUM->SBUF (one op per psum tile = 2 nets)
                        relu_t = []
                        for pr in range(4):
                            h_in = ps[pr][:]
                            if b1_nonzero:
                                hb = spool.tile([128, 512], F32, tag="hb",
                                                name=f"hb{blk}_{j}_{pr}")
                                nc.vector.tensor_tensor(
                                    out=hb[:], in0=h_in,
                                    in1=b1_sb[:, 2 * pr * M:(2 * pr + 2) * M],
                                    op=mybir.AluOpType.add,
                                )
                                h_in = hb[:]
                            rt = relupool.tile([128, 512], F32, tag="rl",
                                               name=f"rl{blk}_{j}_{pr}")
                            nc.scalar.activation(
                                rt[:], h_in, mybir.ActivationFunctionType.Relu
                            )
                            relu_t.append(rt)
                        # DVE: mult by W2, row-reduce -> logits column
                        # (tensor_tensor_reduce crashes TRN2 hw here; the
                        # scalar_tensor_tensor accum path is equivalent)
                        for k in range(K):
                            pr, half = divmod(k, 2)
                            sc = spool.tile([128, M], F32, tag="sc",
                                            name=f"sc{blk}_{j}_{k}")
                            nc.vector.scalar_tensor_tensor(
                                out=sc[:],
                                in0=relu_t[pr][:, half * M:(half + 1) * M],
                                scalar=0.0,
                                in1=w2_sb[:, k * M:(k + 1) * M],
                                op0=mybir.AluOpType.max,
                                op1=mybir.AluOpType.mult,
                                accum_out=logits[:, k * T_SUB + t: k * T_SUB + t + 1],
                            )
                    else:
                        for k in range(K):
                            pr, half = divmod(k, 2)
                            h_in = ps[pr][:, half * M:(half + 1) * M]
                            if b1_nonzero:
                                hb = spool.tile([128, M], F32, tag="hb",
                                                name=f"hb{blk}_{j}_{k}")
                                nc.vector.tensor_tensor(
                                    out=hb[:], in0=h_in,
                                    in1=b1_sb[:, k * M:(k + 1) * M],
                                    op=mybir.AluOpType.add,
                                )
                                h_in = hb[:]
                            sc = spool.tile([128, M], F32, tag="sc",
                                            name=f"sc{blk}_{j}_{k}")
                            nc.vector.scalar_tensor_tensor(
                                out=sc[:], in0=h_in, scalar=0.0,
                                in1=w2_sb[:, k * M:(k + 1) * M],
                                op0=mybir.AluOpType.max,
                                op1=mybir.AluOpType.mult,
                                accum_out=logits[:, k * T_SUB + t: k * T_SUB + t + 1],
                            )

            # epilogue: gate -> sigmoid / hard threshold, per net
            for k in range(K):
                gate = epool.tile([128, T_SUB], F32, tag="gate")
                nc.vector.tensor_tensor(
                    out=gate[:],
                    in0=logits[:, k * T_SUB:(k + 1) * T_SUB],
                    in1=gn_sb[:, k * T_SUB:(k + 1) * T_SUB],
                    op=mybir.AluOpType.add,
                )
                ew_t = epool.tile([128, T_SUB], F32, tag="ew")
                nc.scalar.activation(
                    ew_t[:], gate[:], mybir.ActivationFunctionType.Sigmoid
                )
                hd_t = epool.tile([128, T_SUB], F32, tag="hd")
                nc.vector.tensor_scalar(
                    out=hd_t[:], in0=gate[:], scalar1=0.0, scalar2=None,
                    op0=mybir.AluOpType.is_ge,
                )
                nc.sync.dma_start(ew_d[k], ew_t[:])
                nc.sync.dma_start(hd_d[k], hd_t[:])

    nc.compile()
    return nc


def _run(in_maps, b1_nonzero, use_f32r):
    nc = _build_program(b1_nonzero, use_f32r)
    return run_bass_kernel_spmd(
        nc, in_maps, list(range(N_CORES)), trace=TRACE, tmpdir=TRACE_DIR
    )


def _fixup(ew, hd, triplet_emb, gate_noise, W1, b1, W2, b2):
    """Recompute near-threshold edges in exact fp32 on the host."""
    for k in range(K):
        es = np.where(np.abs(ew[k] - 0.5) <= FIXUP_EW_DELTA)[0]
        if es.size == 0:
            continue
        h = np.maximum(triplet_emb[es] @ W1[k] + b1[k], np.float32(0.0))
        lg = h @ W2[k] + b2[k]
        gate = gate_noise[k, es] + lg
        ew32 = np.float32(1.0) / (np.float32(1.0) + np.exp(-gate, dtype=np.float32))
        ew[k, es] = ew32
        hd[k, es] = (ew32 >= np.float32(0.5)).astype(np.float32)


def kernel(triplet_emb, noise, W1, b1, W2, b2, edge_index=None):
    global LAST_RESULTS
    triplet_emb = np.asarray(triplet_emb, dtype=np.float32)
    noise = np.asarray(noise, dtype=np.float32)
    W1 = np.asarray(W1, dtype=np.float32)
    b1 = np.asarray(b1, dtype=np.float32)
    W2 = np.asarray(W2, dtype=np.float32)
    b2 = np.asarray(b2, dtype=np.float32)

    # ---- host-side layout prep ----
    # gate noise (elementwise log terms, fp32 to match the reference) + b2
    eps = (1.0 - BIAS) - (1.0 - 2.0 * BIAS) * noise
    gate_noise = (np.log(eps) - np.log1p(-eps)).astype(np.float32)
    gn = gate_noise + b2[:, None]
    gn_pad = np.zeros((K, E_PAD), np.float32)
    gn_pad[:, :E] = gn

    # W1 packed as [c, pair, 128f, 512] with nets 2pr|2pr+1 side by side
    w1p = np.ascontiguousarray(
        W1.reshape(4, 2, 3, 128, M).transpose(2, 0, 3, 1, 4).reshape(3, 4, 128, 512)
    )
    w2r = np.ascontiguousarray(np.broadcast_to(W2.reshape(1, K * M), (128, K * M)))
    b1_nonzero = bool(np.any(b1))
    b1r = (
        np.ascontiguousarray(np.broadcast_to(b1.reshape(1, K * M), (128, K * M)))
        if b1_nonzero
        else None
    )

    x_pad = np.zeros((E_PAD, F), np.float32)
    x_pad[:E] = triplet_emb

    in_maps = []
    for c in range(N_CORES):
        lo, hi = c * E_CORE, (c + 1) * E_CORE
        xt_c = np.ascontiguousarray(x_pad[lo:hi].T).reshape(3, 128, E_CORE)
        gn_c = np.ascontiguousarray(
            gn_pad[:, lo:hi].reshape(K, T_SUB, 128).transpose(0, 2, 1)
        )
        m = {"xt": xt_c, "w1p": w1p, "w2r": w2r, "gn": gn_c}
        if b1_nonzero:
            m["b1r"] = b1r
        in_maps.append(m)

    use_f32r = USE_F32R
    try:
        results = _run(in_maps, b1_nonzero, use_f32r)
    except Exception:
        if not use_f32r:
            raise
        use_f32r = False
        results = _run(in_maps, b1_nonzero, use_f32r)
    LAST_RESULTS = results

    ew = np.empty((K, E_PAD), np.float32)
    hd = np.empty((K, E_PAD), np.float32)
    for c in range(N_CORES):
        lo, hi = c * E_CORE, (c + 1) * E_CORE
        ew[:, lo:hi] = results.results[c]["ew"].transpose(0, 2, 1).reshape(K, E_CORE)
        hd[:, lo:hi] = results.results[c]["hd"].transpose(0, 2, 1).reshape(K, E_CORE)

    edge_weight = np.ascontiguousarray(ew[:, :E])
    hard_edge_weight = np.ascontiguousarray(hd[:, :E])
    if use_f32r:
        _fixup(edge_weight, hard_edge_weight,
               triplet_emb, gate_noise, W1, b1, W2, b2)
    return edge_weight, hard_edge_weight
